# revision 18
# baseline (speedup 1.0000x reference)
"""AttentiveReadout Trainium2 kernel (8-core SPMD, data-parallel over graphs).

Math: for each graph g (128 nodes each, nodes sorted by graph):
  scores[n,h] = x[n] @ Ws[:,h]            (Ws folds key_w and query; key_b drops
                                           out of the softmax; a 9th all-zero
                                           "head" yields exact uniform weights
                                           for the mean-pool path)
  e = exp(scores)                         (softmax shift invariant; |s| < ~1)
  A[g,h,:] = sum_n e[n,h] * x[n,:]        (unnormalized weighted feature sums)
  pooled[g] = blockdiag_h(A[g,h,:]/Z @ value_w.T) + value_b
  ctx = pooled @ out_w.T + out_b ; avg = A[g,8,:]/128
  gate = sigmoid([ctx,avg] @ gate_w.T + gate_b)
  out = LayerNorm(gate*ctx + (1-gate)*avg) * ln_w + ln_b

The big win: values = x @ value_w.T (68.7 GFLOP) is never computed; the
contraction is reordered through the per-graph A sums, so per-node work is
just the [512,9] scores GEMM plus rank-9 per-graph matmuls.  x is shipped in
bf16 in BOTH layouts (node-major for the A-matmuls, feature-major for the
scores GEMM) so no on-chip transpose of x is needed.  fp32 everywhere past
the A sums (accumulation is always fp32 in PSUM).
"""
import numpy as np
import ml_dtypes

N_NODES = 131072
IN_F = 512
H = 8
DH = 64
B = 1024
SEG = 128          # nodes per graph
N_CORES = 8
GPC = B // N_CORES     # graphs per core = 128
NPC = N_NODES // N_CORES   # nodes per core = 16384
LN_EPS = 1e-5

_CACHE = {}
_ABLATE = set()  # timeline-sim ablation flags (profiling only)


def _build_nc(G, NB=4, repeat=1):
    """Build the single-core Bass program for G graphs (NB graphs per block).

    repeat>1 wraps the whole body in a For_i that re-runs it (identical
    output) — used only for timing, to amortize the dispatch overhead."""
    import concourse.mybir as mybir
    import concourse.tile as tile
    import concourse.bacc as bacc

    f32 = mybir.dt.float32
    bf16 = mybir.dt.bfloat16
    AFT = mybir.ActivationFunctionType
    AX = mybir.AxisListType

    NBLK = G // NB
    NN = G * SEG  # nodes this core

    nc = bacc.Bacc("TRN2", target_bir_lowering=False, debug=False,
                   num_devices=N_CORES)
    x_bf = nc.declare_dram_parameter("x_bf", [NN, 512], bf16, isOutput=False)
    xt_bf = nc.declare_dram_parameter("xt_bf", [512, NN], bf16, isOutput=False)
    ws = nc.declare_dram_parameter("ws", [128, 36], bf16, isOutput=False)
    i9 = nc.declare_dram_parameter("i9", [9, 9], bf16, isOutput=False)
    i128 = nc.declare_dram_parameter("i128", [128, 128], f32, isOutput=False)
    vw = nc.declare_dram_parameter("vw", [128, 2048], f32, isOutput=False)
    ow = nc.declare_dram_parameter("ow", [128, 2048], f32, isOutput=False)
    gw = nc.declare_dram_parameter("gw", [128, 4096], f32, isOutput=False)
    sel = nc.declare_dram_parameter("sel", [9, 512], f32, isOutput=False)
    ob2 = nc.declare_dram_parameter("ob2", [128, 4], f32, isOutput=False)
    gb2 = nc.declare_dram_parameter("gb2", [128, 4], f32, isOutput=False)
    lnw = nc.declare_dram_parameter("lnw", [1, 512], f32, isOutput=False)
    lnb = nc.declare_dram_parameter("lnb", [1, 512], f32, isOutput=False)
    ones1 = nc.declare_dram_parameter("ones1", [1, 128], f32, isOutput=False)
    out = nc.declare_dram_parameter("out", [G, 512], f32, isOutput=True)

    with tile.TileContext(nc) as tc:
        from contextlib import ExitStack
        with ExitStack() as octx:
            if repeat > 1:
                octx.enter_context(tc.For_i(
                    0, repeat, 1,
                    hint_engines=(mybir.EngineType.PE, mybir.EngineType.DVE,
                                  mybir.EngineType.Activation,
                                  mybir.EngineType.Pool, mybir.EngineType.SP)))
            ctx = octx
            cpool = ctx.enter_context(tc.tile_pool(name="const", bufs=1))
            ws_sb = cpool.tile([128, 36], bf16, tag="ws")
            nc.gpsimd.dma_start(ws_sb[:], ws[:])
            i9_sb = cpool.tile([9, 9], bf16, tag="i9")
            nc.gpsimd.dma_start(i9_sb[:], i9[:])

            apool = ctx.enter_context(tc.tile_pool(name="accum", bufs=1))
            # A^T for all graphs: column g*36 + fb*9 + h  (f = fb*128 + p)
            at_sb = apool.tile([128, G * 36], f32, tag="at")
            z_sb = apool.tile([9, G], f32, tag="z")

            # ---------------- phase 1: per-graph sums ----------------
            with tc.tile_pool(name="xin", bufs=3) as xpool, \
                 tc.tile_pool(name="xtin", bufs=3) as xtpool, \
                 tc.tile_pool(name="esb", bufs=4) as epool, \
                 tc.tile_pool(name="etsb", bufs=4) as etpool, \
                 tc.tile_pool(name="sps", bufs=2, space="PSUM") as spspool, \
                 tc.tile_pool(name="etps", bufs=2, space="PSUM") as etpspool, \
                 tc.tile_pool(name="atps", bufs=3, space="PSUM") as atpspool:
                for blk in range(NBLK):
                    xb = xpool.tile([128, NB * 512], bf16, tag="xb")
                    xtb = xtpool.tile([128, 4 * NB * 128], bf16, tag="xtb")
                    if "tinydma" in _ABLATE:
                        nc.gpsimd.dma_start(xb[:, 0:8], x_bf[0:128, 0:8])
                        nc.gpsimd.dma_start(xtb[:, 0:8], xt_bf[0:128, 0:8])
                    else:
                        nc.gpsimd.dma_start(
                            xb[:],
                            x_bf[blk * NB * SEG:(blk + 1) * NB * SEG, :]
                            .rearrange("(b p) f -> p b f", p=128))
                        nc.gpsimd.dma_start(
                            xtb[:],
                            xt_bf[:, blk * NB * SEG:(blk + 1) * NB * SEG]
                            .rearrange("(fb p) n -> p fb n", p=128))

                    # scores^T for NB graphs: [9, NB*128]
                    sps = spspool.tile([9, NB * 128], f32, tag="sps")
                    for fb in range(1 if "noscores" in _ABLATE else 4):
                        nc.tensor.matmul(
                            sps[:], ws_sb[:, fb * 9:(fb + 1) * 9],
                            xtb[:, fb * NB * 128:(fb + 1) * NB * 128],
                            start=(fb == 0), stop=(fb == 3 or "noscores" in _ABLATE))

                    for b in range(NB):
                        g = blk * NB + b
                        # e = exp(scores), Z accumulated per (head, graph)
                        et = epool.tile([9, 128], bf16, tag="et")
                        nc.scalar.activation(
                            et[:], sps[:, b * 128:(b + 1) * 128], AFT.Exp,
                            accum_out=z_sb[:, g:g + 1])
                        # transpose e^T -> e [128 nodes, 9]
                        etp = etpspool.tile([128, 9], bf16, tag="etp")
                        nc.tensor.transpose(etp[:], et[:], i9_sb[:])
                        esb = etpool.tile([128, 9], bf16, tag="e")
                        nc.vector.tensor_copy(esb[:], etp[:])
                        # A^T block: for each f-block, x_blk.T @ e -> [128f, 9]
                        atp = atpspool.tile([128, 36], f32, tag="atp")
                        for fb in range(1 if "noamm" in _ABLATE else 4):
                            nc.tensor.matmul(
                                atp[:, fb * 9:(fb + 1) * 9],
                                xb[:, b * 512 + fb * 128: b * 512 + (fb + 1) * 128],
                                esb[:], start=(fb == 0), stop=(fb == 3 or "noamm" in _ABLATE))
                        nc.vector.tensor_copy(at_sb[:, g * 36:(g + 1) * 36], atp[:])

            # ---------------- phase 2: per-graph readout ----------------
            at_r = at_sb[:].rearrange("p (g w) -> p w g", w=36)
            with tc.tile_pool(name="p2", bufs=1) as p2, \
                 tc.tile_pool(name="p2w", bufs=1) as p2w, \
                 tc.tile_pool(name="p2ps", bufs=4, space="PSUM") as p2ps:
                vw_sb = p2w.tile([128, 2048], f32, tag="vw")
                nc.gpsimd.dma_start(vw_sb[:], vw[:])
                ow_sb = p2w.tile([128, 2048], f32, tag="ow")
                nc.gpsimd.dma_start(ow_sb[:], ow[:])
                gw_sb = p2w.tile([128, 4096], f32, tag="gw")
                nc.gpsimd.dma_start(gw_sb[:], gw[:])
                sel_sb = p2w.tile([9, 512], f32, tag="sel")
                nc.gpsimd.dma_start(sel_sb[:], sel[:])
                ob2_sb = p2w.tile([128, 4], f32, tag="ob2")
                nc.gpsimd.dma_start(ob2_sb[:], ob2[:])
                gb2_sb = p2w.tile([128, 4], f32, tag="gb2")
                nc.gpsimd.dma_start(gb2_sb[:], gb2[:])
                lnw_sb = p2w.tile([1, 512], f32, tag="lnw")
                nc.gpsimd.dma_start(lnw_sb[:], lnw[:])
                lnb_sb = p2w.tile([1, 512], f32, tag="lnb")
                nc.gpsimd.dma_start(lnb_sb[:], lnb[:])
                ones1_sb = p2w.tile([1, 128], f32, tag="ones1")
                nc.gpsimd.dma_start(ones1_sb[:], ones1[:])
                i128_sb = p2w.tile([128, 128], f32, tag="i128")
                nc.gpsimd.dma_start(i128_sb[:], i128[:])

                # rz = 1/Z ; rz_big[c_local + 128*pb, g] = rz[head(c), g]
                rz = p2.tile([9, G], f32, tag="rz")
                nc.vector.reciprocal(rz[:], z_sb[:])
                rz_big = p2.tile([128, 4 * G], f32, tag="rzbig")
                for pb in range(4):
                    rzp = p2ps.tile([128, G], f32, tag="p2t")
                    nc.tensor.matmul(rzp[:],
                                     sel_sb[:, pb * 128:(pb + 1) * 128],
                                     rz[:], start=True, stop=True)
                    nc.vector.tensor_copy(rz_big[:, pb * G:(pb + 1) * G], rzp[:])

                # pooled^T [c, g] (unnormalized), then normalize by rz_big
                pooled = p2.tile([128, 4 * G], f32, tag="pooled")
                for h in range(8):
                    pps = p2ps.tile([64, G], f32, tag="p2t")
                    for kc in range(4):
                        nc.tensor.matmul(
                            pps[:],
                            vw_sb[:, kc * 512 + h * 64: kc * 512 + (h + 1) * 64],
                            at_r[:, kc * 9 + h, :],
                            start=(kc == 0), stop=(kc == 3))
                    hp, base = h // 2, (h % 2) * 64
                    nc.vector.tensor_copy(
                        pooled[base:base + 64, hp * G:(hp + 1) * G], pps[:])
                nc.vector.tensor_mul(pooled[:], pooled[:], rz_big[:])

                # avg^T (normalized by exact 1/128)
                avgn = p2.tile([128, 4 * G], f32, tag="avgn")
                for pb in range(4):
                    nc.scalar.mul(avgn[:, pb * G:(pb + 1) * G],
                                  at_r[:, pb * 9 + 8, :], 1.0 / SEG)

                # ctx^T = out_w @ pooled^T + out_b'   [4pb x 128, G]
                ctxt = p2.tile([128, 4 * G], f32, tag="ctxt")
                for mb in range(4):
                    cps = p2ps.tile([128, G], f32, tag="p2t")
                    for kc in range(4):
                        nc.tensor.matmul(
                            cps[:],
                            ow_sb[:, kc * 512 + mb * 128: kc * 512 + (mb + 1) * 128],
                            pooled[:, kc * G:(kc + 1) * G],
                            start=(kc == 0), stop=(kc == 3))
                    nc.scalar.activation(ctxt[:, mb * G:(mb + 1) * G], cps[:],
                                         AFT.Identity, bias=ob2_sb[:, mb:mb + 1])

                # gate^T = sigmoid(gate_w @ [ctx; avg] + gate_b)
                gate = p2.tile([128, 4 * G], f32, tag="gate")
                for mb in range(4):
                    gps = p2ps.tile([128, G], f32, tag="p2t")
                    for kc in range(8):
                        rhs = (ctxt[:, kc * G:(kc + 1) * G] if kc < 4
                               else avgn[:, (kc - 4) * G:(kc - 3) * G])
                        nc.tensor.matmul(
                            gps[:],
                            gw_sb[:, kc * 512 + mb * 128: kc * 512 + (mb + 1) * 128],
                            rhs, start=(kc == 0), stop=(kc == 7))
                    nc.scalar.activation(gate[:, mb * G:(mb + 1) * G], gps[:],
                                         AFT.Sigmoid, bias=gb2_sb[:, mb:mb + 1])

                # emb^T = avgn + gate*(ctx - avgn)
                d = p2.tile([128, 4 * G], f32, tag="d")
                nc.vector.tensor_sub(d[:], ctxt[:], avgn[:])
                nc.vector.tensor_mul(d[:], gate[:], d[:])
                embt = p2.tile([128, 4 * G], f32, tag="embt")
                nc.vector.tensor_add(embt[:], avgn[:], d[:])

                # transpose emb^T -> emb [G, 512]
                emb = p2.tile([G, 512], f32, tag="emb")
                for pb in range(4):
                    tps = p2ps.tile([G, 128], f32, tag="p2t")
                    nc.tensor.transpose(tps[:], embt[:, pb * G:(pb + 1) * G],
                                        i128_sb[:])
                    nc.vector.tensor_copy(emb[:, pb * 128:(pb + 1) * 128], tps[:])

                # LayerNorm over features (free dim)
                mu = p2.tile([G, 1], f32, tag="mu")
                nc.vector.reduce_sum(mu[:], emb[:], axis=AX.X)
                nc.scalar.mul(mu[:], mu[:], -1.0 / 512)
                cent = p2.tile([G, 512], f32, tag="cent")
                nc.vector.tensor_scalar_add(cent[:], emb[:], mu[:])
                sq = p2.tile([G, 512], f32, tag="sq")
                vs = p2.tile([G, 1], f32, tag="vs")
                nc.scalar.activation(sq[:], cent[:], AFT.Square, accum_out=vs[:])
                eps = p2.tile([G, 1], f32, tag="eps")
                nc.vector.memset(eps[:], LN_EPS)
                sd = p2.tile([G, 1], f32, tag="sd")
                nc.scalar.activation(sd[:], vs[:], AFT.Sqrt,
                                     bias=eps[:], scale=1.0 / 512)
                rstd = p2.tile([G, 1], f32, tag="rstd")
                nc.vector.reciprocal(rstd[:], sd[:])
                nc.vector.tensor_scalar_mul(cent[:], cent[:], rstd[:])

                # * ln_w + ln_b (broadcast along partitions via K=1 matmuls)
                lw_ps = p2ps.tile([128, 512], f32, tag="p2t")
                nc.tensor.matmul(lw_ps[:], ones1_sb[:], lnw_sb[:],
                                 start=True, stop=True)
                lw_b = p2.tile([128, 512], f32, tag="lwb")
                nc.vector.tensor_copy(lw_b[:], lw_ps[:])
                lb_ps = p2ps.tile([128, 512], f32, tag="p2t")
                nc.tensor.matmul(lb_ps[:], ones1_sb[:], lnb_sb[:],
                                 start=True, stop=True)
                lb_b = p2.tile([128, 512], f32, tag="lbb")
                nc.vector.tensor_copy(lb_b[:], lb_ps[:])

                res = p2.tile([G, 512], f32, tag="res")
                nc.vector.tensor_mul(res[:], cent[:], lw_b[0:G, :])
                nc.vector.tensor_add(res[:], res[:], lb_b[0:G, :])
                nc.gpsimd.dma_start(out[:], res[:])
    nc.compile()
    return nc


def _prep_weights(query, key_w, value_w, out_w, out_b, value_b, gate_w,
                  gate_b, ln_w, ln_b):
    bf16 = ml_dtypes.bfloat16
    F = IN_F
    # scores weight: Ws[f,h] = sum_d key_w[h*64+d, f]*query[h,d]; col 8 = 0
    Ws = np.zeros((F, 9), np.float32)
    Ws[:, :H] = (key_w.reshape(H, DH, F) * query[:, :, None]).sum(1).T
    ws_r = np.zeros((128, 36), np.float32)
    for fb in range(4):
        ws_r[:, fb * 9:(fb + 1) * 9] = Ws[fb * 128:(fb + 1) * 128, :]
    com = {
        "ws": ws_r.astype(bf16),
        "i9": np.eye(9, dtype=bf16),
        "i128": np.eye(128, dtype=np.float32),
        "vw": np.ascontiguousarray(
            value_w.T.reshape(4, 128, 512).transpose(1, 0, 2).reshape(128, 2048)),
        "ow": np.ascontiguousarray(
            out_w.T.reshape(4, 128, 512).transpose(1, 0, 2).reshape(128, 2048)),
        "gw": np.ascontiguousarray(
            gate_w.T.reshape(8, 128, 512).transpose(1, 0, 2).reshape(128, 4096)),
        "ob2": np.ascontiguousarray((out_b + out_w @ value_b).reshape(4, 128).T),
        "gb2": np.ascontiguousarray(gate_b.reshape(4, 128).T),
        "lnw": ln_w.reshape(1, 512).astype(np.float32),
        "lnb": ln_b.reshape(1, 512).astype(np.float32),
        "ones1": np.ones((1, 128), np.float32),
    }
    sel = np.zeros((9, 512), np.float32)
    for pb in range(4):
        for c in range(128):
            sel[2 * pb + c // 64, pb * 128 + c] = 1.0
    com["sel"] = sel
    return {k: np.ascontiguousarray(v) for k, v in com.items()}


def _reference_np(x, batch, query, key_w, key_b, value_w, value_b, out_w,
                  out_b, gate_w, gate_b, ln_w, ln_b):
    """Safety-net numpy fallback for unexpected (non-uniform) batch layouts."""
    N = x.shape[0]
    nb = int(batch.max()) + 1
    keys = (x @ key_w.T + key_b).reshape(N, H, DH)
    scores = np.einsum('nhd,hd->nh', keys, query)
    smax = np.full((nb, H), -np.inf, np.float32)
    np.maximum.at(smax, batch, scores)
    smax = np.where(np.isfinite(smax), smax, 0.0)
    e = np.exp(scores - smax[batch])
    ssum = np.zeros((nb, H), np.float32)
    np.add.at(ssum, batch, e)
    w = e / np.maximum(ssum[batch], 1e-12)
    values = (x @ value_w.T + value_b).reshape(N, H, DH)
    pooled = np.zeros((nb, H, DH), np.float32)
    np.add.at(pooled, batch, w[:, :, None] * values)
    ctx = pooled.reshape(nb, H * DH) @ out_w.T + out_b
    counts = np.zeros((nb,), np.float32)
    np.add.at(counts, batch, np.ones((N,), np.float32))
    avg = np.zeros((nb, x.shape[1]), np.float32)
    np.add.at(avg, batch, x)
    avg = avg / np.maximum(counts, 1.0)[:, None]
    gate = 1.0 / (1.0 + np.exp(-(np.concatenate([ctx, avg], 1) @ gate_w.T + gate_b)))
    ctx = gate * ctx + (1.0 - gate) * avg
    emb = np.where(counts[:, None] > 0, ctx, 0.0)
    mu = emb.mean(-1, keepdims=True)
    var = emb.var(-1, keepdims=True)
    return ((emb - mu) / np.sqrt(var + LN_EPS) * ln_w + ln_b).astype(np.float32)


def _make_inmaps(x, query, key_w, value_w, value_b, out_w, out_b, gate_w,
                 gate_b, ln_w, ln_b):
    bf16 = ml_dtypes.bfloat16
    x_bf = x.astype(bf16)
    xt_bf = np.ascontiguousarray(x_bf.T)
    com = _prep_weights(query, key_w, value_w, out_w, out_b, value_b, gate_w,
                        gate_b, ln_w, ln_b)
    in_maps = []
    for k in range(N_CORES):
        m = dict(com)
        m["x_bf"] = x_bf[k * NPC:(k + 1) * NPC]
        m["xt_bf"] = np.ascontiguousarray(xt_bf[:, k * NPC:(k + 1) * NPC])
        in_maps.append(m)
    return in_maps


def _make_exec_fn(nc, in_maps):
    """Build a non-donating jitted executor over 8 cores with device-resident
    inputs.  Returns (fn, dev_args)."""
    import jax
    import numpy as np
    from jax.sharding import Mesh, PartitionSpec, NamedSharding
    from jax.experimental.shard_map import shard_map
    from concourse import bass2jax, mybir

    part_name = (nc.partition_id_tensor.name
                 if nc.partition_id_tensor else None)
    in_names, out_names, out_avals, zero_outs = [], [], [], []
    for alloc in nc.m.functions[0].allocations:
        if not isinstance(alloc, mybir.MemoryLocationSet):
            continue
        name = alloc.memorylocations[0].name
        if alloc.kind == "ExternalInput":
            if name != part_name:
                in_names.append(name)
        elif alloc.kind == "ExternalOutput":
            out_names.append(name)
            dt_np = mybir.dt.np(alloc.dtype)
            out_avals.append(jax.core.ShapedArray(
                tuple(alloc.tensor_shape), dt_np))
            zero_outs.append(np.zeros(tuple(alloc.tensor_shape), dt_np))
    n_params = len(in_names)
    all_in_names = list(in_names) + list(out_names)
    if part_name is not None:
        all_in_names.append(part_name)

    def _body(*params):
        operands = list(params)
        if part_name is not None:
            operands.append(bass2jax.partition_id_tensor())
        outs = bass2jax._bass_exec_p.bind(
            *operands,
            out_avals=tuple(out_avals),
            in_names=tuple(all_in_names),
            out_names=tuple(out_names),
            lowering_input_output_aliases=(),
            sim_require_finite=True,
            sim_require_nnan=True,
            nc=nc)
        return tuple(outs)

    devices = jax.devices()[:N_CORES]
    mesh = Mesh(np.array(devices), ("core",))
    spec = PartitionSpec("core")
    n_outs = len(out_avals)
    fn = jax.jit(shard_map(_body, mesh=mesh,
                           in_specs=(spec,) * (n_params + n_outs),
                           out_specs=(spec,) * n_outs, check_rep=False),
                 keep_unused=True)
    sh = NamedSharding(mesh, spec)
    dev_args = [jax.device_put(
                    np.concatenate([np.asarray(m[nm]) for m in in_maps], 0), sh)
                for nm in in_names]
    dev_args += [jax.device_put(
                    np.zeros((N_CORES * z.shape[0], *z.shape[1:]), z.dtype), sh)
                 for z in zero_outs]
    return fn, dev_args


def _time_exec(fn, dev_args, reps):
    import jax, time
    outs = fn(*dev_args)
    jax.block_until_ready(outs)
    best = float("inf")
    for _ in range(3):
        t0 = time.perf_counter()
        res = [fn(*dev_args) for _ in range(reps)]
        jax.block_until_ready(res)
        best = min(best, (time.perf_counter() - t0) / reps)
    return best


def _build_trivial_nc():
    """Tiny kernel for launch-overhead calibration."""
    import concourse.mybir as mybir
    import concourse.tile as tile
    import concourse.bacc as bacc
    f32 = mybir.dt.float32
    nc = bacc.Bacc("TRN2", target_bir_lowering=False, debug=False,
                   num_devices=N_CORES)
    a = nc.declare_dram_parameter("a", [128, 128], f32, isOutput=False)
    o = nc.declare_dram_parameter("o", [128, 128], f32, isOutput=True)
    with tile.TileContext(nc) as tc:
        with tc.tile_pool(name="sb", bufs=1) as sb:
            t = sb.tile([128, 128], f32)
            nc.gpsimd.dma_start(t[:], a[:])
            nc.gpsimd.dma_start(o[:], t[:])
    nc.compile()
    return nc


def profile_hw_ns(inputs, r_lo=8, r_hi=72):
    """True per-execution HW time via repeat-loop slope: build the kernel
    wrapped in a For_i that runs the body R times per dispatch, measure wall
    time at two R values through identical dispatch paths, and difference.
    Returns ns per kernel body execution (includes ~6us loop back-edge)."""
    try:
        import numpy as np
        args = [np.asarray(inputs[k], np.float32) for k in
                ("query", "key_w", "value_w", "value_b", "out_w", "out_b",
                 "gate_w", "gate_b", "ln_w", "ln_b")]
        (query, key_w, value_w, value_b, out_w, out_b, gate_w, gate_b,
         ln_w, ln_b) = args
        in_maps = _make_inmaps(np.asarray(inputs["x"], np.float32), query,
                               key_w, value_w, value_b, out_w, out_b,
                               gate_w, gate_b, ln_w, ln_b)
        ts = {}
        for r in (r_lo, r_hi):
            key = f"nc_rep{r}"
            nc = _CACHE.get(key)
            if nc is None:
                nc = _CACHE[key] = _build_nc(GPC, repeat=r)
            fn, dev_args = _make_exec_fn(nc, in_maps)
            ts[r] = _time_exec(fn, dev_args, reps=8)
            print(f"[profile] R={r}: wall/call={ts[r]*1e6:.1f}us")
        return int((ts[r_hi] - ts[r_lo]) / (r_hi - r_lo) * 1e9)
    except Exception:
        import traceback
        traceback.print_exc()
        return None


def kernel(x, batch, query, key_w, key_b, value_w, value_b, out_w, out_b,
           gate_w, gate_b, ln_w, ln_b):
    x = np.asarray(x, np.float32)
    batch = np.asarray(batch)
    args = [np.asarray(a, np.float32) for a in
            (query, key_w, key_b, value_w, value_b, out_w, out_b, gate_w,
             gate_b, ln_w, ln_b)]
    (query, key_w, key_b, value_w, value_b, out_w, out_b, gate_w, gate_b,
     ln_w, ln_b) = args

    exp_batch = (np.arange(N_NODES) // SEG).astype(batch.dtype)
    if x.shape != (N_NODES, IN_F) or not np.array_equal(batch, exp_batch):
        return _reference_np(x, batch, query, key_w, key_b, value_w, value_b,
                             out_w, out_b, gate_w, gate_b, ln_w, ln_b)

    from concourse.bass_utils import run_bass_kernel_spmd

    if "nc" not in _CACHE:
        _CACHE["nc"] = _build_nc(GPC)
    nc = _CACHE["nc"]

    in_maps = _make_inmaps(x, query, key_w, value_w, value_b, out_w, out_b,
                           gate_w, gate_b, ln_w, ln_b)
    res = run_bass_kernel_spmd(nc, in_maps, list(range(N_CORES)))
    return np.concatenate([res.results[k]["out"] for k in range(N_CORES)], 0)


# revision 21
# speedup vs baseline: 1.1999x; 1.1999x over previous
"""AttentiveReadout Trainium2 kernel (8-core SPMD, data-parallel over graphs).

Math: for each graph g (128 nodes each, nodes sorted by graph):
  scores[n,h] = x[n] @ Ws[:,h]            (Ws folds key_w and query; key_b drops
                                           out of the softmax; a 9th all-zero
                                           "head" yields exact uniform weights
                                           for the mean-pool path)
  e = exp(scores)                         (softmax shift invariant; |s| < ~1)
  A[g,h,:] = sum_n e[n,h] * x[n,:]        (unnormalized weighted feature sums)
  pooled[g] = blockdiag_h(A[g,h,:]/Z @ value_w.T) + value_b
  ctx = pooled @ out_w.T + out_b ; avg = A[g,8,:]/128
  gate = sigmoid([ctx,avg] @ gate_w.T + gate_b)
  out = LayerNorm(gate*ctx + (1-gate)*avg) * ln_w + ln_b

The big win: values = x @ value_w.T (68.7 GFLOP) is never computed; the
contraction is reordered through the per-graph A sums, so per-node work is
just the [512,9] scores GEMM plus rank-9 per-graph matmuls.  x is shipped in
bf16 in BOTH layouts (node-major for the A-matmuls, feature-major for the
scores GEMM) so no on-chip transpose of x is needed.  fp32 everywhere past
the A sums (accumulation is always fp32 in PSUM).
"""
import numpy as np
import ml_dtypes

N_NODES = 131072
IN_F = 512
H = 8
DH = 64
B = 1024
SEG = 128          # nodes per graph
N_CORES = 8
GPC = B // N_CORES     # graphs per core = 128
NPC = N_NODES // N_CORES   # nodes per core = 16384
LN_EPS = 1e-5

_CACHE = {}
_ABLATE = set()  # timeline-sim ablation flags (profiling only)


def _build_nc(G, NB=4, repeat=1):
    """Build the single-core Bass program for G graphs (NB graphs per block).

    repeat>1 wraps the whole body in a For_i that re-runs it (identical
    output) — used only for timing, to amortize the dispatch overhead."""
    import concourse.mybir as mybir
    import concourse.tile as tile
    import concourse.bacc as bacc

    f32 = mybir.dt.float32
    bf16 = mybir.dt.bfloat16
    AFT = mybir.ActivationFunctionType
    AX = mybir.AxisListType

    NBLK = G // NB
    NN = G * SEG  # nodes this core

    nc = bacc.Bacc("TRN2", target_bir_lowering=False, debug=False,
                   num_devices=N_CORES)
    x_bf = nc.declare_dram_parameter("x_bf", [NN, 512], bf16, isOutput=False)
    xt_bf = nc.declare_dram_parameter("xt_bf", [512, NN], bf16, isOutput=False)
    ws = nc.declare_dram_parameter("ws", [128, 36], bf16, isOutput=False)
    i9 = nc.declare_dram_parameter("i9", [9, 9], f32, isOutput=False)
    i128 = nc.declare_dram_parameter("i128", [128, 128], f32, isOutput=False)
    vw = nc.declare_dram_parameter("vw", [128, 2048], f32, isOutput=False)
    ow = nc.declare_dram_parameter("ow", [128, 2048], f32, isOutput=False)
    gw = nc.declare_dram_parameter("gw", [128, 4096], f32, isOutput=False)
    sel = nc.declare_dram_parameter("sel", [9, 512], f32, isOutput=False)
    ob2 = nc.declare_dram_parameter("ob2", [128, 4], f32, isOutput=False)
    gb2 = nc.declare_dram_parameter("gb2", [128, 4], f32, isOutput=False)
    lnw = nc.declare_dram_parameter("lnw", [1, 512], f32, isOutput=False)
    lnb = nc.declare_dram_parameter("lnb", [1, 512], f32, isOutput=False)
    ones1 = nc.declare_dram_parameter("ones1", [1, 128], f32, isOutput=False)
    out = nc.declare_dram_parameter("out", [G, 512], f32, isOutput=True)

    with tile.TileContext(nc) as tc:
        from contextlib import ExitStack
        with ExitStack() as octx:
            if repeat > 1:
                octx.enter_context(tc.For_i(
                    0, repeat, 1,
                    hint_engines=(mybir.EngineType.PE, mybir.EngineType.DVE,
                                  mybir.EngineType.Activation,
                                  mybir.EngineType.Pool, mybir.EngineType.SP)))
            ctx = octx
            cpool = ctx.enter_context(tc.tile_pool(name="const", bufs=1))
            ws_sb = cpool.tile([128, 36], bf16, tag="ws")
            nc.gpsimd.dma_start(ws_sb[:], ws[:])
            i9_sb = cpool.tile([9, 9], f32, tag="i9")
            nc.gpsimd.dma_start(i9_sb[:], i9[:])

            apool = ctx.enter_context(tc.tile_pool(name="accum", bufs=1))
            # A^T for all graphs: column g*36 + fb*9 + h  (f = fb*128 + p)
            at_sb = apool.tile([128, G * 36], f32, tag="at")
            z_sb = apool.tile([9, G], f32, tag="z")

            # ---------------- phase 1: per-graph sums ----------------
            with tc.tile_pool(name="xin", bufs=3) as xpool, \
                 tc.tile_pool(name="xtin", bufs=3) as xtpool, \
                 tc.tile_pool(name="esb", bufs=4) as epool, \
                 tc.tile_pool(name="etsb", bufs=4) as etpool, \
                 tc.tile_pool(name="sps", bufs=2, space="PSUM") as spspool, \
                 tc.tile_pool(name="etps", bufs=2, space="PSUM") as etpspool, \
                 tc.tile_pool(name="atps", bufs=3, space="PSUM") as atpspool:
                for blk in range(NBLK):
                    xb = xpool.tile([128, NB * 512], bf16, tag="xb")
                    xtb = xtpool.tile([128, 4 * NB * 128], bf16, tag="xtb")
                    if "tinydma" in _ABLATE:
                        nc.gpsimd.dma_start(xb[:, 0:8], x_bf[0:128, 0:8])
                        nc.gpsimd.dma_start(xtb[:, 0:8], xt_bf[0:128, 0:8])
                    else:
                        nc.gpsimd.dma_start(
                            xb[:],
                            x_bf[blk * NB * SEG:(blk + 1) * NB * SEG, :]
                            .rearrange("(b p) f -> p b f", p=128))
                        nc.gpsimd.dma_start(
                            xtb[:],
                            xt_bf[:, blk * NB * SEG:(blk + 1) * NB * SEG]
                            .rearrange("(fb p) n -> p fb n", p=128))

                    # scores^T for NB graphs: [9, NB*128]
                    sps = spspool.tile([9, NB * 128], f32, tag="sps")
                    for fb in range(1 if "noscores" in _ABLATE else 4):
                        nc.tensor.matmul(
                            sps[:], ws_sb[:, fb * 9:(fb + 1) * 9],
                            xtb[:, fb * NB * 128:(fb + 1) * NB * 128],
                            start=(fb == 0), stop=(fb == 3 or "noscores" in _ABLATE))

                    # one exp for the whole block: e^T [9, NB*128] fp32
                    et = epool.tile([9, NB * 128], f32, tag="et")
                    nc.scalar.activation(et[:], sps[:], AFT.Exp)
                    # Z[h, g] per block via segmented reduce over nodes
                    nc.vector.reduce_sum(
                        z_sb[:, blk * NB:(blk + 1) * NB],
                        et[:].rearrange("p (b n) -> p b n", n=128),
                        axis=AX.X)
                    # transpose each graph's e^T -> e [128, 9], batched in one
                    # PSUM tile + one copy
                    etp = etpspool.tile([128, NB * 9], f32, tag="etp")
                    for b in range(NB):
                        nc.tensor.transpose(etp[:, b * 9:(b + 1) * 9],
                                            et[:, b * 128:(b + 1) * 128],
                                            i9_sb[:])
                    esb = etpool.tile([128, NB * 9], bf16, tag="e")
                    nc.vector.tensor_copy(esb[:], etp[:])
                    # A^T: per (graph, f-block) matmul into one shared PSUM
                    # tile, one copy per block
                    atp = atpspool.tile([128, NB * 36], f32, tag="atp")
                    for b in range(NB):
                        nfb = 1 if "noamm" in _ABLATE else 4
                        for fb in range(nfb):
                            nc.tensor.matmul(
                                atp[:, b * 36 + fb * 9: b * 36 + (fb + 1) * 9],
                                xb[:, b * 512 + fb * 128: b * 512 + (fb + 1) * 128],
                                esb[:, b * 9:(b + 1) * 9],
                                start=(fb == 0), stop=(fb == nfb - 1))
                    nc.vector.tensor_copy(
                        at_sb[:, blk * NB * 36:(blk + 1) * NB * 36], atp[:])

            # ---------------- phase 2: per-graph readout ----------------
            at_r = at_sb[:].rearrange("p (g w) -> p w g", w=36)
            with tc.tile_pool(name="p2", bufs=1) as p2, \
                 tc.tile_pool(name="p2w", bufs=1) as p2w, \
                 tc.tile_pool(name="p2ps", bufs=4, space="PSUM") as p2ps:
                vw_sb = p2w.tile([128, 2048], f32, tag="vw")
                nc.gpsimd.dma_start(vw_sb[:], vw[:])
                ow_sb = p2w.tile([128, 2048], f32, tag="ow")
                nc.gpsimd.dma_start(ow_sb[:], ow[:])
                gw_sb = p2w.tile([128, 4096], f32, tag="gw")
                nc.gpsimd.dma_start(gw_sb[:], gw[:])
                sel_sb = p2w.tile([9, 512], f32, tag="sel")
                nc.gpsimd.dma_start(sel_sb[:], sel[:])
                ob2_sb = p2w.tile([128, 4], f32, tag="ob2")
                nc.gpsimd.dma_start(ob2_sb[:], ob2[:])
                gb2_sb = p2w.tile([128, 4], f32, tag="gb2")
                nc.gpsimd.dma_start(gb2_sb[:], gb2[:])
                lnw_sb = p2w.tile([1, 512], f32, tag="lnw")
                nc.gpsimd.dma_start(lnw_sb[:], lnw[:])
                lnb_sb = p2w.tile([1, 512], f32, tag="lnb")
                nc.gpsimd.dma_start(lnb_sb[:], lnb[:])
                ones1_sb = p2w.tile([1, 128], f32, tag="ones1")
                nc.gpsimd.dma_start(ones1_sb[:], ones1[:])
                i128_sb = p2w.tile([128, 128], f32, tag="i128")
                nc.gpsimd.dma_start(i128_sb[:], i128[:])

                # rz = 1/Z ; rz_big[c_local + 128*pb, g] = rz[head(c), g]
                rz = p2.tile([9, G], f32, tag="rz")
                nc.vector.reciprocal(rz[:], z_sb[:])
                rz_big = p2.tile([128, 4 * G], f32, tag="rzbig")
                for pb in range(4):
                    rzp = p2ps.tile([128, G], f32, tag="p2t")
                    nc.tensor.matmul(rzp[:],
                                     sel_sb[:, pb * 128:(pb + 1) * 128],
                                     rz[:], start=True, stop=True)
                    nc.vector.tensor_copy(rz_big[:, pb * G:(pb + 1) * G], rzp[:])

                # pooled^T [c, g] (unnormalized), then normalize by rz_big
                pooled = p2.tile([128, 4 * G], f32, tag="pooled")
                for h in range(8):
                    pps = p2ps.tile([64, G], f32, tag="p2t")
                    for kc in range(4):
                        nc.tensor.matmul(
                            pps[:],
                            vw_sb[:, kc * 512 + h * 64: kc * 512 + (h + 1) * 64],
                            at_r[:, kc * 9 + h, :],
                            start=(kc == 0), stop=(kc == 3))
                    hp, base = h // 2, (h % 2) * 64
                    nc.vector.tensor_copy(
                        pooled[base:base + 64, hp * G:(hp + 1) * G], pps[:])
                nc.vector.tensor_mul(pooled[:], pooled[:], rz_big[:])

                # avg^T (normalized by exact 1/128)
                avgn = p2.tile([128, 4 * G], f32, tag="avgn")
                for pb in range(4):
                    nc.scalar.mul(avgn[:, pb * G:(pb + 1) * G],
                                  at_r[:, pb * 9 + 8, :], 1.0 / SEG)

                # ctx^T = out_w @ pooled^T + out_b'   [4pb x 128, G]
                ctxt = p2.tile([128, 4 * G], f32, tag="ctxt")
                for mb in range(4):
                    cps = p2ps.tile([128, G], f32, tag="p2t")
                    for kc in range(4):
                        nc.tensor.matmul(
                            cps[:],
                            ow_sb[:, kc * 512 + mb * 128: kc * 512 + (mb + 1) * 128],
                            pooled[:, kc * G:(kc + 1) * G],
                            start=(kc == 0), stop=(kc == 3))
                    nc.scalar.activation(ctxt[:, mb * G:(mb + 1) * G], cps[:],
                                         AFT.Identity, bias=ob2_sb[:, mb:mb + 1])

                # gate^T = sigmoid(gate_w @ [ctx; avg] + gate_b)
                gate = p2.tile([128, 4 * G], f32, tag="gate")
                for mb in range(4):
                    gps = p2ps.tile([128, G], f32, tag="p2t")
                    for kc in range(8):
                        rhs = (ctxt[:, kc * G:(kc + 1) * G] if kc < 4
                               else avgn[:, (kc - 4) * G:(kc - 3) * G])
                        nc.tensor.matmul(
                            gps[:],
                            gw_sb[:, kc * 512 + mb * 128: kc * 512 + (mb + 1) * 128],
                            rhs, start=(kc == 0), stop=(kc == 7))
                    nc.scalar.activation(gate[:, mb * G:(mb + 1) * G], gps[:],
                                         AFT.Sigmoid, bias=gb2_sb[:, mb:mb + 1])

                # emb^T = avgn + gate*(ctx - avgn)
                d = p2.tile([128, 4 * G], f32, tag="d")
                nc.vector.tensor_sub(d[:], ctxt[:], avgn[:])
                nc.vector.tensor_mul(d[:], gate[:], d[:])
                embt = p2.tile([128, 4 * G], f32, tag="embt")
                nc.vector.tensor_add(embt[:], avgn[:], d[:])

                # transpose emb^T -> emb [G, 512]
                emb = p2.tile([G, 512], f32, tag="emb")
                for pb in range(4):
                    tps = p2ps.tile([G, 128], f32, tag="p2t")
                    nc.tensor.transpose(tps[:], embt[:, pb * G:(pb + 1) * G],
                                        i128_sb[:])
                    nc.vector.tensor_copy(emb[:, pb * 128:(pb + 1) * 128], tps[:])

                # LayerNorm over features (free dim)
                mu = p2.tile([G, 1], f32, tag="mu")
                nc.vector.reduce_sum(mu[:], emb[:], axis=AX.X)
                nc.scalar.mul(mu[:], mu[:], -1.0 / 512)
                cent = p2.tile([G, 512], f32, tag="cent")
                nc.vector.tensor_scalar_add(cent[:], emb[:], mu[:])
                sq = p2.tile([G, 512], f32, tag="sq")
                vs = p2.tile([G, 1], f32, tag="vs")
                nc.scalar.activation(sq[:], cent[:], AFT.Square, accum_out=vs[:])
                eps = p2.tile([G, 1], f32, tag="eps")
                nc.vector.memset(eps[:], LN_EPS)
                sd = p2.tile([G, 1], f32, tag="sd")
                nc.scalar.activation(sd[:], vs[:], AFT.Sqrt,
                                     bias=eps[:], scale=1.0 / 512)
                rstd = p2.tile([G, 1], f32, tag="rstd")
                nc.vector.reciprocal(rstd[:], sd[:])
                nc.vector.tensor_scalar_mul(cent[:], cent[:], rstd[:])

                # * ln_w + ln_b (broadcast along partitions via K=1 matmuls)
                lw_ps = p2ps.tile([128, 512], f32, tag="p2t")
                nc.tensor.matmul(lw_ps[:], ones1_sb[:], lnw_sb[:],
                                 start=True, stop=True)
                lw_b = p2.tile([128, 512], f32, tag="lwb")
                nc.vector.tensor_copy(lw_b[:], lw_ps[:])
                lb_ps = p2ps.tile([128, 512], f32, tag="p2t")
                nc.tensor.matmul(lb_ps[:], ones1_sb[:], lnb_sb[:],
                                 start=True, stop=True)
                lb_b = p2.tile([128, 512], f32, tag="lbb")
                nc.vector.tensor_copy(lb_b[:], lb_ps[:])

                res = p2.tile([G, 512], f32, tag="res")
                nc.vector.tensor_mul(res[:], cent[:], lw_b[0:G, :])
                nc.vector.tensor_add(res[:], res[:], lb_b[0:G, :])
                nc.gpsimd.dma_start(out[:], res[:])
    nc.compile()
    return nc


def _prep_weights(query, key_w, value_w, out_w, out_b, value_b, gate_w,
                  gate_b, ln_w, ln_b):
    bf16 = ml_dtypes.bfloat16
    F = IN_F
    # scores weight: Ws[f,h] = sum_d key_w[h*64+d, f]*query[h,d]; col 8 = 0
    Ws = np.zeros((F, 9), np.float32)
    Ws[:, :H] = (key_w.reshape(H, DH, F) * query[:, :, None]).sum(1).T
    ws_r = np.zeros((128, 36), np.float32)
    for fb in range(4):
        ws_r[:, fb * 9:(fb + 1) * 9] = Ws[fb * 128:(fb + 1) * 128, :]
    com = {
        "ws": ws_r.astype(bf16),
        "i9": np.eye(9, dtype=np.float32),
        "i128": np.eye(128, dtype=np.float32),
        "vw": np.ascontiguousarray(
            value_w.T.reshape(4, 128, 512).transpose(1, 0, 2).reshape(128, 2048)),
        "ow": np.ascontiguousarray(
            out_w.T.reshape(4, 128, 512).transpose(1, 0, 2).reshape(128, 2048)),
        "gw": np.ascontiguousarray(
            gate_w.T.reshape(8, 128, 512).transpose(1, 0, 2).reshape(128, 4096)),
        "ob2": np.ascontiguousarray((out_b + out_w @ value_b).reshape(4, 128).T),
        "gb2": np.ascontiguousarray(gate_b.reshape(4, 128).T),
        "lnw": ln_w.reshape(1, 512).astype(np.float32),
        "lnb": ln_b.reshape(1, 512).astype(np.float32),
        "ones1": np.ones((1, 128), np.float32),
    }
    sel = np.zeros((9, 512), np.float32)
    for pb in range(4):
        for c in range(128):
            sel[2 * pb + c // 64, pb * 128 + c] = 1.0
    com["sel"] = sel
    return {k: np.ascontiguousarray(v) for k, v in com.items()}


def _reference_np(x, batch, query, key_w, key_b, value_w, value_b, out_w,
                  out_b, gate_w, gate_b, ln_w, ln_b):
    """Safety-net numpy fallback for unexpected (non-uniform) batch layouts."""
    N = x.shape[0]
    nb = int(batch.max()) + 1
    keys = (x @ key_w.T + key_b).reshape(N, H, DH)
    scores = np.einsum('nhd,hd->nh', keys, query)
    smax = np.full((nb, H), -np.inf, np.float32)
    np.maximum.at(smax, batch, scores)
    smax = np.where(np.isfinite(smax), smax, 0.0)
    e = np.exp(scores - smax[batch])
    ssum = np.zeros((nb, H), np.float32)
    np.add.at(ssum, batch, e)
    w = e / np.maximum(ssum[batch], 1e-12)
    values = (x @ value_w.T + value_b).reshape(N, H, DH)
    pooled = np.zeros((nb, H, DH), np.float32)
    np.add.at(pooled, batch, w[:, :, None] * values)
    ctx = pooled.reshape(nb, H * DH) @ out_w.T + out_b
    counts = np.zeros((nb,), np.float32)
    np.add.at(counts, batch, np.ones((N,), np.float32))
    avg = np.zeros((nb, x.shape[1]), np.float32)
    np.add.at(avg, batch, x)
    avg = avg / np.maximum(counts, 1.0)[:, None]
    gate = 1.0 / (1.0 + np.exp(-(np.concatenate([ctx, avg], 1) @ gate_w.T + gate_b)))
    ctx = gate * ctx + (1.0 - gate) * avg
    emb = np.where(counts[:, None] > 0, ctx, 0.0)
    mu = emb.mean(-1, keepdims=True)
    var = emb.var(-1, keepdims=True)
    return ((emb - mu) / np.sqrt(var + LN_EPS) * ln_w + ln_b).astype(np.float32)


def _make_inmaps(x, query, key_w, value_w, value_b, out_w, out_b, gate_w,
                 gate_b, ln_w, ln_b):
    bf16 = ml_dtypes.bfloat16
    x_bf = x.astype(bf16)
    xt_bf = np.ascontiguousarray(x_bf.T)
    com = _prep_weights(query, key_w, value_w, out_w, out_b, value_b, gate_w,
                        gate_b, ln_w, ln_b)
    in_maps = []
    for k in range(N_CORES):
        m = dict(com)
        m["x_bf"] = x_bf[k * NPC:(k + 1) * NPC]
        m["xt_bf"] = np.ascontiguousarray(xt_bf[:, k * NPC:(k + 1) * NPC])
        in_maps.append(m)
    return in_maps


def _make_exec_fn(nc, in_maps):
    """Build a non-donating jitted executor over 8 cores with device-resident
    inputs.  Returns (fn, dev_args)."""
    import jax
    import numpy as np
    from jax.sharding import Mesh, PartitionSpec, NamedSharding
    from jax.experimental.shard_map import shard_map
    from concourse import bass2jax, mybir

    part_name = (nc.partition_id_tensor.name
                 if nc.partition_id_tensor else None)
    in_names, out_names, out_avals, zero_outs = [], [], [], []
    for alloc in nc.m.functions[0].allocations:
        if not isinstance(alloc, mybir.MemoryLocationSet):
            continue
        name = alloc.memorylocations[0].name
        if alloc.kind == "ExternalInput":
            if name != part_name:
                in_names.append(name)
        elif alloc.kind == "ExternalOutput":
            out_names.append(name)
            dt_np = mybir.dt.np(alloc.dtype)
            out_avals.append(jax.core.ShapedArray(
                tuple(alloc.tensor_shape), dt_np))
            zero_outs.append(np.zeros(tuple(alloc.tensor_shape), dt_np))
    n_params = len(in_names)
    all_in_names = list(in_names) + list(out_names)
    if part_name is not None:
        all_in_names.append(part_name)

    def _body(*params):
        operands = list(params)
        if part_name is not None:
            operands.append(bass2jax.partition_id_tensor())
        outs = bass2jax._bass_exec_p.bind(
            *operands,
            out_avals=tuple(out_avals),
            in_names=tuple(all_in_names),
            out_names=tuple(out_names),
            lowering_input_output_aliases=(),
            sim_require_finite=True,
            sim_require_nnan=True,
            nc=nc)
        return tuple(outs)

    devices = jax.devices()[:N_CORES]
    mesh = Mesh(np.array(devices), ("core",))
    spec = PartitionSpec("core")
    n_outs = len(out_avals)
    fn = jax.jit(shard_map(_body, mesh=mesh,
                           in_specs=(spec,) * (n_params + n_outs),
                           out_specs=(spec,) * n_outs, check_rep=False),
                 keep_unused=True)
    sh = NamedSharding(mesh, spec)
    dev_args = [jax.device_put(
                    np.concatenate([np.asarray(m[nm]) for m in in_maps], 0), sh)
                for nm in in_names]
    dev_args += [jax.device_put(
                    np.zeros((N_CORES * z.shape[0], *z.shape[1:]), z.dtype), sh)
                 for z in zero_outs]
    return fn, dev_args


def _time_exec(fn, dev_args, reps):
    import jax, time
    outs = fn(*dev_args)
    jax.block_until_ready(outs)
    best = float("inf")
    for _ in range(3):
        t0 = time.perf_counter()
        res = [fn(*dev_args) for _ in range(reps)]
        jax.block_until_ready(res)
        best = min(best, (time.perf_counter() - t0) / reps)
    return best


def _build_trivial_nc():
    """Tiny kernel for launch-overhead calibration."""
    import concourse.mybir as mybir
    import concourse.tile as tile
    import concourse.bacc as bacc
    f32 = mybir.dt.float32
    nc = bacc.Bacc("TRN2", target_bir_lowering=False, debug=False,
                   num_devices=N_CORES)
    a = nc.declare_dram_parameter("a", [128, 128], f32, isOutput=False)
    o = nc.declare_dram_parameter("o", [128, 128], f32, isOutput=True)
    with tile.TileContext(nc) as tc:
        with tc.tile_pool(name="sb", bufs=1) as sb:
            t = sb.tile([128, 128], f32)
            nc.gpsimd.dma_start(t[:], a[:])
            nc.gpsimd.dma_start(o[:], t[:])
    nc.compile()
    return nc


def profile_hw_ns(inputs, r_lo=8, r_hi=72):
    """True per-execution HW time via repeat-loop slope: build the kernel
    wrapped in a For_i that runs the body R times per dispatch, measure wall
    time at two R values through identical dispatch paths, and difference.
    Returns ns per kernel body execution (includes ~6us loop back-edge)."""
    try:
        import numpy as np
        args = [np.asarray(inputs[k], np.float32) for k in
                ("query", "key_w", "value_w", "value_b", "out_w", "out_b",
                 "gate_w", "gate_b", "ln_w", "ln_b")]
        (query, key_w, value_w, value_b, out_w, out_b, gate_w, gate_b,
         ln_w, ln_b) = args
        in_maps = _make_inmaps(np.asarray(inputs["x"], np.float32), query,
                               key_w, value_w, value_b, out_w, out_b,
                               gate_w, gate_b, ln_w, ln_b)
        ts = {}
        for r in (r_lo, r_hi):
            key = f"nc_rep{r}"
            nc = _CACHE.get(key)
            if nc is None:
                nc = _CACHE[key] = _build_nc(GPC, repeat=r)
            fn, dev_args = _make_exec_fn(nc, in_maps)
            ts[r] = _time_exec(fn, dev_args, reps=8)
            print(f"[profile] R={r}: wall/call={ts[r]*1e6:.1f}us")
        return int((ts[r_hi] - ts[r_lo]) / (r_hi - r_lo) * 1e9)
    except Exception:
        import traceback
        traceback.print_exc()
        return None


def kernel(x, batch, query, key_w, key_b, value_w, value_b, out_w, out_b,
           gate_w, gate_b, ln_w, ln_b):
    x = np.asarray(x, np.float32)
    batch = np.asarray(batch)
    args = [np.asarray(a, np.float32) for a in
            (query, key_w, key_b, value_w, value_b, out_w, out_b, gate_w,
             gate_b, ln_w, ln_b)]
    (query, key_w, key_b, value_w, value_b, out_w, out_b, gate_w, gate_b,
     ln_w, ln_b) = args

    exp_batch = (np.arange(N_NODES) // SEG).astype(batch.dtype)
    if x.shape != (N_NODES, IN_F) or not np.array_equal(batch, exp_batch):
        return _reference_np(x, batch, query, key_w, key_b, value_w, value_b,
                             out_w, out_b, gate_w, gate_b, ln_w, ln_b)

    from concourse.bass_utils import run_bass_kernel_spmd

    if "nc" not in _CACHE:
        _CACHE["nc"] = _build_nc(GPC)
    nc = _CACHE["nc"]

    in_maps = _make_inmaps(x, query, key_w, value_w, value_b, out_w, out_b,
                           gate_w, gate_b, ln_w, ln_b)
    res = run_bass_kernel_spmd(nc, in_maps, list(range(N_CORES)))
    return np.concatenate([res.results[k]["out"] for k in range(N_CORES)], 0)
